# revision 6
# baseline (speedup 1.0000x reference)
"""Bass/Tile TRN2 kernel for the MeanFieldGaussianLayer loss.

reference math:
    mean  = tensor[:, :, 0]                       (B, T)
    f_var = softplus(tensor[:, :, 1])
    y_var = f_var + softplus(noise) + 1e-6
    logp  = -0.5 * sum_T(LOG_2PI + log(y_var) + (y - mean)^2 / y_var)
    out   = mean_B(logp)

Strategy: pure data-parallel over B across 8 cores, 64 rows/core, staged as
three bf16 planes [128, 8192]: t1, y, nt0 = -t0.  Per tile:

    DMA (HWDGE, one queue, line rate): t1, nt0, y tiles
    ACT pass 1:   u = Exp(t1)                    (bf16 -> bf16)
    ACT pass 2:   v = Ln(a*u + a) = y_var        (fp32 out)
    sub:          d = y + nt0                    (DVE TT bf16 2x / GpSimd)
    S1 term:      split between ACT pass 3 (exact Ln + accum) and DVE
                  MEGA1 (deg-3 lsq poly of ln(v), 1 op) to balance engines
    S2 term:      DVE MEGA2: accum += d^2 * recip_nr1(v)   (1 op)

recip_nr1 = BITWISE_NOT exponent-flip seed + 1 inline Newton step (~0.17%).
Host adds LOG_2PI and the poly constant term, sums partials in fp64.
"""

import os
import sys

import numpy as np

if "/opt/trn_rl_repo" not in sys.path:
    sys.path.insert(0, "/opt/trn_rl_repo")

import ml_dtypes

import concourse.bass as bass
import concourse.tile as tile
from concourse import bacc, mybir
from concourse import bass_utils

BF16 = ml_dtypes.bfloat16

# ---------------------------------------------------------------------------
# Patch 1: force all ACT functions into the one table set that contains
# Exp+Ln, so no per-tile ACT_TABLE_LOAD flip-flop (~1.3us each).
# ---------------------------------------------------------------------------
import concourse.bacc as _bacc_mod

_ACT_KEEP = "natural_log_exp_and_others"
_ACT_STRIP = {
    mybir.ActivationFunctionType.Exp,
    mybir.ActivationFunctionType.Ln,
    mybir.ActivationFunctionType.Square,
}
_orig_get_tables = _bacc_mod.get_activation_tables


def _patched_get_tables(arch):
    tabs = _orig_get_tables(arch)
    return {
        name: (set(fns) if name == _ACT_KEEP else set(fns) - _ACT_STRIP)
        for name, fns in tabs.items()
    }


_bacc_mod.get_activation_tables = _patched_get_tables

# ---------------------------------------------------------------------------
# Patch 1b: cheaper Tile kernel tail (drop the trailing all-engine barrier).
# ---------------------------------------------------------------------------
import concourse.tile as _tile_mod
from concourse.vector_clock import ScopedClock as _ScopedClock


def _cheap_drain_and_barrier(self, tick_clock, wait_clock):
    drain_inst = self.nc.sync.drain()
    wait_clock.add_sem_waits(
        drain_inst.ins, _ScopedClock({None: tick_clock.global_clock})
    )
    self.nc.all_engine_barrier()
    popped = self.nc._tile_sem_poison_stack.pop()
    assert popped is self._sem_poison
    self.nc.clear_and_free_semaphores(list(self.sems.allocated().values()))


_tile_mod.TileContext._drain_and_barrier = _cheap_drain_and_barrier

# ---------------------------------------------------------------------------
# Patch 2: custom fused DVE ops.
#   MEGA1_LNPOLY_ANT: out = ((C0*v + C1)*v + C2)*v        ; accum += out
#   MEGA2_D2R_ANT:    r = NOT-seed+NR1 recip(v);
#                     out = Src1^2 * r                    ; accum += out
# ---------------------------------------------------------------------------
import concourse.dve_ops as _dve_ops
from concourse.dve_ops import DveOp
from concourse.dve_spec import (
    AluOp,
    Bin,
    C0,
    C1,
    C2,
    Spec,
    Src0,
    Src1,
    Zero,
    _has_src1,
    lower,
    sq,
)
from concourse.dve_uop import DveOpSpec
from operator import add as _op_add


def _register(name, spec):
    if name in _dve_ops._SUB_OPCODE_FOR_NAME:
        return next(op for op in _dve_ops.OPS if op.name == name)
    row = max(_dve_ops._SUB_OPCODE_FOR_NAME.values()) + 1
    assert row < 0x20
    shas = {}
    for ver in ("v3", "v4"):
        try:
            uops = lower(spec, ver=ver)
            shas[ver] = DveOpSpec(
                name=name, opcode=row, uops=uops, rd1_en=_has_src1(spec)
            ).sha(ver)
        except Exception:
            pass
    op = DveOp(name, spec, subdim=False, uops_sha=shas)
    _dve_ops._SUB_OPCODE_FOR_NAME[name] = row
    _dve_ops.OPS.append(op)
    _dve_ops.CUSTOM_DVE_SPECS[name] = spec
    return op


MEGA1 = _register(
    "MEGA1_LNPOLY_ANT",
    Spec(
        body=((C0 * Src0 + C1) * Src0 + C2) * Src0,
        accum=_op_add,
        accum_init=Zero,
        reference=lambda in0, in1, c0, c1, c2: (
            ((c0 * in0 + c1) * in0 + c2) * in0
        ),
    ),
)

_nx = Bin(AluOp.BITWISE_NOT, Src0, Src0)
_y0 = _nx * C0
_y1 = _y0 * (C1 - Src0 * _y0)


def _ref_mega2(in0, in1, c0, c1, c2):
    nx = (~np.asarray(in0, np.float32).view(np.int32)).view(np.float32)
    y0 = nx * np.float32(c0)
    y1 = y0 * (np.float32(c1) - np.asarray(in0, np.float32) * y0)
    return np.square(np.asarray(in1, np.float32)) * y1


MEGA2 = _register(
    "MEGA2_D2R_ANT",
    Spec(body=sq(Src1) * _y1, accum=_op_add, accum_init=Zero, reference=_ref_mega2),
)

SEED_C0 = -0.23549792
SEED_C1 = 2.0017324

B, T = 512, 16384
NCORES = 8
ROWS = B // NCORES          # 64 rows per core
P = 128                     # SBUF partitions
FPP = ROWS * T // P         # 8192 elems per partition per plane
FDS = [2048, 2048, 2048, 1024, 1024]
assert sum(FDS) == FPP
NT = len(FDS)
# Engine split per tile:
#   sub_eng[k]:  'v' = DVE tensor add (bf16 2x), 'g' = GpSimd
#   s1_act[k]:   elems of the tile whose S1 uses exact ACT Ln+accum (prefix);
#                the rest go through DVE MEGA1.
SUB_ENG = ['v', 'g', 'g', 'v', 'v']
S1_ACT = [0, 2048, 1024, 0, 0]

LOG_2PI = float(np.log(2.0 * np.pi))
JITTER = 1e-6
C_DEFAULT = float(np.log(2.0)) + JITTER
# deg-3 lsq fit of v -> ln(softplus(t1)+c) over t1~N(0,1) through the bf16
# staging + bf16 Exp pipeline: [c3, c2, c1, c0]
LNPOLY_DEFAULT = (0.04594413, -0.43504742, 1.68584316, -1.29956715)

_BUILD_CACHE: dict[float, object] = {}
_POLY_CACHE: dict[float, tuple] = {}
LAST_RESULT = None  # BassKernelResults of the most recent run (for test harness)


def _lnpoly_for(c: float) -> tuple:
    """deg-3 lsq fit of kernel-v -> ln(v_ref) for noise offset c."""
    if abs(c - C_DEFAULT) < 1e-12:
        return LNPOLY_DEFAULT
    got = _POLY_CACHE.get(c)
    if got is not None:
        return got
    a = float(np.exp(c))
    rng = np.random.default_rng(123)
    t1 = rng.standard_normal(2_000_000).astype(np.float32)
    t1b = t1.astype(BF16).astype(np.float32)
    u = np.exp(t1b).astype(BF16).astype(np.float32)
    v = np.log(a * u + a).astype(np.float32)
    v_ref = np.log1p(np.exp(-np.abs(t1))) + np.maximum(t1, 0) + c
    A = np.stack([v**3, v**2, v, np.ones_like(v)], axis=1).astype(np.float64)
    coef, *_ = np.linalg.lstsq(A, np.log(v_ref.astype(np.float64)), rcond=None)
    out = tuple(float(x) for x in coef)
    _POLY_CACHE[c] = out
    return out


def _build(a: float, lnpoly: tuple):
    """Build + compile the SPMD program. `a` = exp(softplus(noise) + jitter)."""
    f32 = mybir.dt.float32
    b16 = mybir.dt.bfloat16
    Act = mybir.ActivationFunctionType
    c3, c2, c1, _c0 = lnpoly

    nc = bacc.Bacc("TRN2", target_bir_lowering=False, debug=False)

    t1 = nc.dram_tensor("t1", [P, FPP], b16, kind="ExternalInput").ap()
    nt0 = nc.dram_tensor("nt0", [P, FPP], b16, kind="ExternalInput").ap()
    y = nc.dram_tensor("y", [P, FPP], b16, kind="ExternalInput").ap()
    out = nc.dram_tensor("out", [P, 3], f32, kind="ExternalOutput").ap()

    offs = [0]
    for FD in FDS:
        offs.append(offs[-1] + FD)
    sls = [slice(offs[i], offs[i + 1]) for i in range(NT)]

    with tile.TileContext(nc) as tc:
        with (
            tc.tile_pool(name="io", bufs=1) as io,
            tc.tile_pool(name="mid", bufs=2) as mid,
            tc.tile_pool(name="accs", bufs=1) as accs,
        ):
            acc_a = accs.tile([P, NT], f32)    # S1 partials via ACT Ln accum
            acc_m = accs.tile([P, NT], f32)    # S1 partials via MEGA1 poly
            acc_p = accs.tile([P, NT], f32)    # S2 partials via MEGA2
            # not every column gets written (per-tile engine split) -> zero
            # them so the final reduce over all NT columns is safe
            nc.vector.memset(acc_a[:], 0.0)
            nc.vector.memset(acc_m[:], 0.0)
            outt = accs.tile([P, 3], f32)
            abias = accs.tile([P, 1], f32)
            nc.vector.memset(abias[:], a)
            zbias = accs.tile([P, 1], f32)
            nc.vector.memset(zbias[:], 0.0)

            # --- DMA issue: single HWDGE FIFO, tile-interleaved by need ---
            t1_t, y_t, n_t = [], [], []
            for k in range(NT):
                tt = io.tile([P, FDS[k]], b16, tag=f"t1_{k}", name=f"t1_{k}")
                nc.sync.dma_start(tt[:], t1[:, sls[k]])
                t1_t.append(tt)
                ty = io.tile([P, FDS[k]], b16, tag=f"y_{k}", name=f"y_{k}")
                nc.sync.dma_start(ty[:], y[:, sls[k]])
                y_t.append(ty)
                tn = io.tile([P, FDS[k]], b16, tag=f"n_{k}", name=f"n_{k}")
                nc.sync.dma_start(tn[:], nt0[:, sls[k]])
                n_t.append(tn)

            # --- compute ---
            for k in range(NT):
                FD = FDS[k]
                u = mid.tile([P, FD], b16, tag="u")
                nc.scalar.activation(u[:], t1_t[k][:], Act.Exp, bias=zbias[:, 0:1])
                v = mid.tile([P, FD], f32, tag="v")
                nc.scalar.activation(
                    v[:], u[:], Act.Ln, bias=abias[:, 0:1], scale=a
                )

                # d = y + (-t0), overwrite the y tile
                d = y_t[k]
                if SUB_ENG[k] == 'g':
                    nc.gpsimd.tensor_add(d[:], y_t[k][:], n_t[k][:])
                else:
                    nc.vector.tensor_add(d[:], y_t[k][:], n_t[k][:])

                # S1 split: exact ACT Ln on [0:na), MEGA1 poly on [na:FD)
                na = S1_ACT[k]
                if na > 0:
                    scr = mid.tile([P, na], b16, tag="scr")
                    nc.scalar.activation(
                        scr[:], v[:, 0:na], Act.Ln, bias=zbias[:, 0:1],
                        accum_out=acc_a[:, k : k + 1],
                    )
                if na < FD:
                    scr1 = mid.tile([P, FD - na], b16, tag="scr1")
                    nc.vector._custom_dve(
                        MEGA1,
                        out=scr1[:],
                        in0=v[:, na:FD],
                        s0=c3, s1=c2, imm2=c1,
                        accum_out=acc_m[:, k : k + 1],
                    )

                scr2 = mid.tile([P, FD], b16, tag="scr2")
                nc.vector._custom_dve(
                    MEGA2,
                    out=scr2[:],
                    in0=v[:],
                    in1=d[:],
                    s0=SEED_C0, s1=SEED_C1, imm2=0.0,
                    accum_out=acc_p[:, k : k + 1],
                )

            nc.vector.reduce_sum(outt[:, 0:1], acc_a[:], axis=mybir.AxisListType.X)
            nc.vector.reduce_sum(outt[:, 1:2], acc_m[:], axis=mybir.AxisListType.X)
            nc.vector.reduce_sum(outt[:, 2:3], acc_p[:], axis=mybir.AxisListType.X)
            nc.sync.dma_start(out[:], outt[:])

    nc.compile()
    return nc


def kernel(tensor, y_target, noise_unconstrained):
    global LAST_RESULT
    noise = np.float64(np.asarray(noise_unconstrained))
    c = float(np.log1p(np.exp(-abs(noise))) + max(noise, 0.0) + JITTER)
    a = float(np.exp(c))
    lnpoly = _lnpoly_for(c)

    key = a
    nc = _BUILD_CACHE.get(key)
    if nc is None:
        nc = _build(a, lnpoly)
        _BUILD_CACHE[key] = nc

    tensor = np.asarray(tensor, dtype=np.float32)
    y_target = np.asarray(y_target, dtype=np.float32)

    in_maps = []
    for k in range(NCORES):
        sh = tensor[k * ROWS : (k + 1) * ROWS]          # (64, 16384, 2)
        in_maps.append(
            {
                "t1": np.ascontiguousarray(sh[:, :, 1]).reshape(P, FPP).astype(BF16),
                "nt0": (-np.ascontiguousarray(sh[:, :, 0]).reshape(P, FPP)).astype(BF16),
                "y": np.ascontiguousarray(
                    y_target[k * ROWS : (k + 1) * ROWS, :, 0]
                ).reshape(P, FPP).astype(BF16),
            }
        )

    trace = os.environ.get("BASS_KERNEL_PROFILE", "0") == "1"
    res = bass_utils.run_bass_kernel_spmd(
        nc, in_maps, list(range(NCORES)), trace=trace
    )
    LAST_RESULT = res

    total = np.float64(0.0)
    for k in range(NCORES):
        o = np.asarray(res.results[k]["out"], dtype=np.float64)
        total += o.sum()
    # constant terms: LOG_2PI everywhere; the poly's constant term c0 for
    # every element whose S1 went through MEGA1.
    mega1_elems = sum(FDS[k] - S1_ACT[k] for k in range(NT))
    total += np.float64(B) * np.float64(T) * np.float64(LOG_2PI)
    total += np.float64(NCORES * P * mega1_elems) * np.float64(lnpoly[3])
    return np.array(-0.5 * total / B, dtype=np.float32)


# revision 12
# speedup vs baseline: 1.0098x; 1.0098x over previous
"""Bass/Tile TRN2 kernel for the MeanFieldGaussianLayer loss.

reference math:
    mean  = tensor[:, :, 0]                       (B, T)
    f_var = softplus(tensor[:, :, 1])
    y_var = f_var + softplus(noise) + 1e-6
    logp  = -0.5 * sum_T(LOG_2PI + log(y_var) + (y - mean)^2 / y_var)
    out   = mean_B(logp)

Strategy: pure data-parallel over B across 8 cores, 64 rows/core, staged as
three bf16 planes [128, 8192]: t1, y, nt0 = -t0.  Per tile:

    DMA (HWDGE, one queue, line rate): t1, nt0, y tiles
    ACT pass 1:   u = Exp(t1)                    (bf16 -> bf16)
    ACT pass 2:   v = Ln(a*u + a) = y_var        (fp32 out)
    sub:          d = y + nt0                    (DVE TT bf16 2x / GpSimd)
    S1 term:      split between ACT pass 3 (exact Ln + accum) and DVE
                  MEGA1 (deg-3 lsq poly of ln(v), 1 op) to balance engines
    S2 term:      DVE MEGA2: accum += d^2 * recip_nr1(v)   (1 op)

recip_nr1 = BITWISE_NOT exponent-flip seed + 1 inline Newton step (~0.17%).
Host adds LOG_2PI and the poly constant term, sums partials in fp64.
"""

import os
import sys

import numpy as np

if "/opt/trn_rl_repo" not in sys.path:
    sys.path.insert(0, "/opt/trn_rl_repo")

import ml_dtypes

import concourse.bass as bass
import concourse.tile as tile
from concourse import bacc, mybir
from concourse import bass_utils

BF16 = ml_dtypes.bfloat16

# ---------------------------------------------------------------------------
# Patch 1: force all ACT functions into the one table set that contains
# Exp+Ln, so no per-tile ACT_TABLE_LOAD flip-flop (~1.3us each).
# ---------------------------------------------------------------------------
import concourse.bacc as _bacc_mod

_ACT_KEEP = "natural_log_exp_and_others"
_ACT_STRIP = {
    mybir.ActivationFunctionType.Exp,
    mybir.ActivationFunctionType.Ln,
    mybir.ActivationFunctionType.Square,
}
_orig_get_tables = _bacc_mod.get_activation_tables


def _patched_get_tables(arch):
    tabs = _orig_get_tables(arch)
    return {
        name: (set(fns) if name == _ACT_KEEP else set(fns) - _ACT_STRIP)
        for name, fns in tabs.items()
    }


_bacc_mod.get_activation_tables = _patched_get_tables

# ---------------------------------------------------------------------------
# Patch 1b: cheaper Tile kernel tail (drop the trailing all-engine barrier).
# ---------------------------------------------------------------------------
import concourse.tile as _tile_mod
from concourse.vector_clock import ScopedClock as _ScopedClock


def _cheap_drain_and_barrier(self, tick_clock, wait_clock):
    drain_inst = self.nc.sync.drain()
    wait_clock.add_sem_waits(
        drain_inst.ins, _ScopedClock({None: tick_clock.global_clock})
    )
    self.nc.all_engine_barrier()
    popped = self.nc._tile_sem_poison_stack.pop()
    assert popped is self._sem_poison
    self.nc.clear_and_free_semaphores(list(self.sems.allocated().values()))


_tile_mod.TileContext._drain_and_barrier = _cheap_drain_and_barrier

# ---------------------------------------------------------------------------
# Patch 2: custom fused DVE ops.
#   MEGA1_LNPOLY_ANT: out = ((C0*v + C1)*v + C2)*v        ; accum += out
#   MEGA2_D2R_ANT:    r = NOT-seed+NR1 recip(v);
#                     out = Src1^2 * r                    ; accum += out
# ---------------------------------------------------------------------------
import concourse.dve_ops as _dve_ops
from concourse.dve_ops import DveOp
from concourse.dve_spec import (
    AluOp,
    Bin,
    C0,
    C1,
    C2,
    Spec,
    Src0,
    Src1,
    Zero,
    _has_src1,
    lower,
    sq,
)
from concourse.dve_uop import DveOpSpec
from operator import add as _op_add


def _register(name, spec):
    if name in _dve_ops._SUB_OPCODE_FOR_NAME:
        return next(op for op in _dve_ops.OPS if op.name == name)
    row = max(_dve_ops._SUB_OPCODE_FOR_NAME.values()) + 1
    assert row < 0x20
    shas = {}
    for ver in ("v3", "v4"):
        try:
            uops = lower(spec, ver=ver)
            shas[ver] = DveOpSpec(
                name=name, opcode=row, uops=uops, rd1_en=_has_src1(spec)
            ).sha(ver)
        except Exception:
            pass
    op = DveOp(name, spec, subdim=False, uops_sha=shas)
    _dve_ops._SUB_OPCODE_FOR_NAME[name] = row
    _dve_ops.OPS.append(op)
    _dve_ops.CUSTOM_DVE_SPECS[name] = spec
    return op


MEGA1 = _register(
    "MEGA1_LNPOLY_ANT",
    Spec(
        body=((C0 * Src0 + C1) * Src0 + C2) * Src0,
        accum=_op_add,
        accum_init=Zero,
        reference=lambda in0, in1, c0, c1, c2: (
            ((c0 * in0 + c1) * in0 + c2) * in0
        ),
    ),
)

_nx = Bin(AluOp.BITWISE_NOT, Src0, Src0)
_y0 = _nx * C0
_y1 = _y0 * (C1 - Src0 * _y0)


def _ref_mega2(in0, in1, c0, c1, c2):
    nx = (~np.asarray(in0, np.float32).view(np.int32)).view(np.float32)
    y0 = nx * np.float32(c0)
    y1 = y0 * (np.float32(c1) - np.asarray(in0, np.float32) * y0)
    return np.square(np.asarray(in1, np.float32)) * y1


MEGA2 = _register(
    "MEGA2_D2R_ANT",
    Spec(body=sq(Src1) * _y1, accum=_op_add, accum_init=Zero, reference=_ref_mega2),
)

SEED_C0 = -0.23549792
SEED_C1 = 2.0017324

B, T = 512, 16384
NCORES = 8
ROWS = B // NCORES          # 64 rows per core
P = 128                     # SBUF partitions
FPP = ROWS * T // P         # 8192 elems per partition per plane
FDS = [512, 1536, 1536, 1536, 1536, 1536]      # compute tiles
assert sum(FDS) == FPP
NT = len(FDS)
# DMA groups: (plane, [tile indices]) in HWDGE FIFO issue order.  t1 leads
# (it gates the long Exp->Ln->mega chain); y/nt0 trail (they gate only
# sub->MEGA2).  First t1 group is small so ACT starts early.
DMA_GROUPS = [
    ("t1", [0]),
    ("t1", [1, 2]),
    ("y", [0, 1]),
    ("nt0", [0, 1]),
    ("y", [2, 3]),
    ("t1", [3, 4, 5]),
    ("nt0", [2, 3]),
    ("y", [4, 5]),
    ("nt0", [4, 5]),
]
# Engine split per tile:
#   sub_eng[k]:  'v' = DVE tensor add (bf16 2x), 'g' = GpSimd
#   s1_act[k]:   elems of the tile whose S1 uses exact ACT Ln+accum (prefix);
#                the rest go through DVE MEGA1.
SUB_ENG = ['g', 'g', 'v', 'v', 'v', 'v']
S1_ACT = [0, 1536, 1536, 0, 0, 0]

LOG_2PI = float(np.log(2.0 * np.pi))
JITTER = 1e-6
C_DEFAULT = float(np.log(2.0)) + JITTER
# deg-3 lsq fit of v -> ln(softplus(t1)+c) over t1~N(0,1) through the bf16
# staging + bf16 Exp pipeline: [c3, c2, c1, c0]
LNPOLY_DEFAULT = (0.04594413, -0.43504742, 1.68584316, -1.29956715)

_BUILD_CACHE: dict[float, object] = {}
_POLY_CACHE: dict[float, tuple] = {}
LAST_RESULT = None  # BassKernelResults of the most recent run (for test harness)


def _lnpoly_for(c: float) -> tuple:
    """deg-3 lsq fit of kernel-v -> ln(v_ref) for noise offset c."""
    if abs(c - C_DEFAULT) < 1e-12:
        return LNPOLY_DEFAULT
    got = _POLY_CACHE.get(c)
    if got is not None:
        return got
    a = float(np.exp(c))
    rng = np.random.default_rng(123)
    t1 = rng.standard_normal(2_000_000).astype(np.float32)
    t1b = t1.astype(BF16).astype(np.float32)
    u = np.exp(t1b).astype(BF16).astype(np.float32)
    v = np.log(a * u + a).astype(np.float32)
    v_ref = np.log1p(np.exp(-np.abs(t1))) + np.maximum(t1, 0) + c
    A = np.stack([v**3, v**2, v, np.ones_like(v)], axis=1).astype(np.float64)
    coef, *_ = np.linalg.lstsq(A, np.log(v_ref.astype(np.float64)), rcond=None)
    out = tuple(float(x) for x in coef)
    _POLY_CACHE[c] = out
    return out


def _build(a: float, lnpoly: tuple):
    """Build + compile the SPMD program. `a` = exp(softplus(noise) + jitter)."""
    f32 = mybir.dt.float32
    b16 = mybir.dt.bfloat16
    Act = mybir.ActivationFunctionType
    c3, c2, c1, _c0 = lnpoly

    nc = bacc.Bacc("TRN2", target_bir_lowering=False, debug=False)

    t1 = nc.dram_tensor("t1", [P, FPP], b16, kind="ExternalInput").ap()
    nt0 = nc.dram_tensor("nt0", [P, FPP], b16, kind="ExternalInput").ap()
    y = nc.dram_tensor("y", [P, FPP], b16, kind="ExternalInput").ap()
    out = nc.dram_tensor("out", [P, 3], f32, kind="ExternalOutput").ap()

    offs = [0]
    for FD in FDS:
        offs.append(offs[-1] + FD)
    sls = [slice(offs[i], offs[i + 1]) for i in range(NT)]

    with tile.TileContext(nc) as tc:
        with (
            tc.tile_pool(name="io", bufs=1) as io,
            tc.tile_pool(name="mid", bufs=2) as mid,
            tc.tile_pool(name="vp", bufs=4) as vp,
            tc.tile_pool(name="accs", bufs=1) as accs,
        ):
            acc_a = accs.tile([P, NT], f32)    # S1 partials via ACT Ln accum
            acc_m = accs.tile([P, NT], f32)    # S1 partials via MEGA1 poly
            acc_p = accs.tile([P, NT], f32)    # S2 partials via MEGA2
            # not every column gets written (per-tile engine split) -> zero
            # them so the final reduce over all NT columns is safe
            nc.vector.memset(acc_a[:], 0.0)
            nc.vector.memset(acc_m[:], 0.0)
            outt = accs.tile([P, 3], f32)
            abias = accs.tile([P, 1], f32)
            nc.vector.memset(abias[:], a)
            zbias = accs.tile([P, 1], f32)
            nc.vector.memset(zbias[:], 0.0)

            # --- DMA issue: single HWDGE FIFO, grouped transfers ---
            planes = {"t1": t1, "y": y, "nt0": nt0}
            # slice views per (plane, compute tile), filled as groups land
            views = {}
            for gi, (pl, tiles) in enumerate(DMA_GROUPS):
                lo, hi = offs[tiles[0]], offs[tiles[-1] + 1]
                gt = io.tile(
                    [P, hi - lo], b16, tag=f"g{gi}", name=f"g{gi}_{pl}"
                )
                nc.sync.dma_start(gt[:], planes[pl][:, lo:hi])
                for k in tiles:
                    views[(pl, k)] = gt[:, offs[k] - lo : offs[k + 1] - lo]
            t1_t = [views[("t1", k)] for k in range(NT)]
            y_t = [views[("y", k)] for k in range(NT)]
            n_t = [views[("nt0", k)] for k in range(NT)]

            # --- compute ---
            for k in range(NT):
                FD = FDS[k]
                u = mid.tile([P, FD], b16, tag="u")
                nc.scalar.activation(u[:], t1_t[k], Act.Exp, bias=zbias[:, 0:1])
                v = vp.tile([P, FD], f32, tag="v")
                nc.scalar.activation(
                    v[:], u[:], Act.Ln, bias=abias[:, 0:1], scale=a
                )

                # S1 split: exact ACT Ln on [0:na), MEGA1 poly on [na:FD).
                # MEGA1 is emitted before the sub so the DVE never head-of-line
                # waits on late y/nt0 data.
                na = S1_ACT[k]
                if na > 0:
                    scr = mid.tile([P, na], b16, tag="scr")
                    nc.scalar.activation(
                        scr[:], v[:, 0:na], Act.Ln, bias=zbias[:, 0:1],
                        accum_out=acc_a[:, k : k + 1],
                    )
                if na < FD:
                    scr1 = mid.tile([P, FD - na], b16, tag="scr1")
                    nc.vector._custom_dve(
                        MEGA1,
                        out=scr1[:],
                        in0=v[:, na:FD],
                        s0=c3, s1=c2, imm2=c1,
                        accum_out=acc_m[:, k : k + 1],
                    )

                # d = y + (-t0), overwrite the y slice in its group tile
                d = y_t[k]
                if SUB_ENG[k] == 'g':
                    nc.gpsimd.tensor_add(d, y_t[k], n_t[k])
                else:
                    nc.vector.tensor_add(d, y_t[k], n_t[k])

                scr2 = mid.tile([P, FD], b16, tag="scr2")
                nc.vector._custom_dve(
                    MEGA2,
                    out=scr2[:],
                    in0=v[:],
                    in1=d,
                    s0=SEED_C0, s1=SEED_C1, imm2=0.0,
                    accum_out=acc_p[:, k : k + 1],
                )

            nc.vector.reduce_sum(outt[:, 0:1], acc_a[:], axis=mybir.AxisListType.X)
            nc.vector.reduce_sum(outt[:, 1:2], acc_m[:], axis=mybir.AxisListType.X)
            nc.vector.reduce_sum(outt[:, 2:3], acc_p[:], axis=mybir.AxisListType.X)
            nc.sync.dma_start(out[:], outt[:])

    nc.compile()
    return nc


def kernel(tensor, y_target, noise_unconstrained):
    global LAST_RESULT
    noise = np.float64(np.asarray(noise_unconstrained))
    c = float(np.log1p(np.exp(-abs(noise))) + max(noise, 0.0) + JITTER)
    a = float(np.exp(c))
    lnpoly = _lnpoly_for(c)

    key = a
    nc = _BUILD_CACHE.get(key)
    if nc is None:
        nc = _build(a, lnpoly)
        _BUILD_CACHE[key] = nc

    tensor = np.asarray(tensor, dtype=np.float32)
    y_target = np.asarray(y_target, dtype=np.float32)

    in_maps = []
    for k in range(NCORES):
        sh = tensor[k * ROWS : (k + 1) * ROWS]          # (64, 16384, 2)
        in_maps.append(
            {
                "t1": np.ascontiguousarray(sh[:, :, 1]).reshape(P, FPP).astype(BF16),
                "nt0": (-np.ascontiguousarray(sh[:, :, 0]).reshape(P, FPP)).astype(BF16),
                "y": np.ascontiguousarray(
                    y_target[k * ROWS : (k + 1) * ROWS, :, 0]
                ).reshape(P, FPP).astype(BF16),
            }
        )

    trace = os.environ.get("BASS_KERNEL_PROFILE", "0") == "1"
    res = bass_utils.run_bass_kernel_spmd(
        nc, in_maps, list(range(NCORES)), trace=trace
    )
    LAST_RESULT = res

    total = np.float64(0.0)
    for k in range(NCORES):
        o = np.asarray(res.results[k]["out"], dtype=np.float64)
        total += o.sum()
    # constant terms: LOG_2PI everywhere; the poly's constant term c0 for
    # every element whose S1 went through MEGA1.
    mega1_elems = sum(FDS[k] - S1_ACT[k] for k in range(NT))
    total += np.float64(B) * np.float64(T) * np.float64(LOG_2PI)
    total += np.float64(NCORES * P * mega1_elems) * np.float64(lnpoly[3])
    return np.array(-0.5 * total / B, dtype=np.float32)
